# revision 1
# baseline (speedup 1.0000x reference)
"""Trainium2 Bass kernel for nn_KB_Mapping_19361712570541 (dense_cnn).

Math (from the reference, with the W=1 image dimension folded away):
  x: [N=131072, C=128]; work in channels-on-partition layout h = x.T [C, N].
  dw3(h, w)[c,n] = w[c,0]*h[c,n-1] + w[c,1]*h[c,n] + w[c,2]*h[c,n+1]   (zero pad)
  b1 = relu(W1pw @ relu(dw3(h, wd1)))
  b2 = (relu(W21x1 @ h) + b1) * mask
  b2 = relu(W2pw @ relu(dw3(b2, wd2)))
  out = relu(Wf[:, :C] @ h + Wf[:, C:] @ b2)          -> out.T is [N, C]

Sharding: data-parallel along N across 8 cores; each core's input slab
carries a 2-column halo of x and a 1-column halo of the mask, so no
cross-core communication is needed (halo intermediates are recomputed).
Mask is zero-padded at the global edges, which exactly reproduces the
reference's zero padding of the second depthwise conv's input.

On-chip: fp16 operands end-to-end (fp32 PSUM accumulation; measured
end-to-end rel err ~5e-4). Pointwise convs are TensorE matmuls (weights
stationary as [I, O]); depthwise 3-taps are diag-matrix matmuls
accumulated in PSUM. The six PSUM->SBUF elementwise materializations
are split DVE {d1 relu, relu+add (STT), d2 relu} / ACT {b1 relu, b2
relu, out relu}; the mask multiply runs on GPSIMD (SBUF-only engine).
Per-tile input/output DMAs (G=1 beat wider chunks once PSUM was tuned).
PSUM: the dw1 accumulator and the fusion accumulator are double-buffered
(2+2+1+1+1+1 = 8 banks); SBUF working tiles 24-deep. Engine busy is
balanced (PE ~95us, ACT ~90us, DVE ~90us per core in the cost-model
timeline; ~101us end-to-end estimate per core). A single narrow
(256-col) leading tile fills the pipeline faster than a uniform grid;
the relu+add and mask-multiply are split into 4B-aligned 1/4-3/4
pieces so GPSIMD's first-piece multiply overlaps VectorE's larger
second-piece relu+add (uneven because a GPSIMD piece runs ~1.4x a
VectorE piece; 50/50 and 3-piece splits are both slower). Exec sits
within ~1% of VectorE's busy time -- the five PSUM->SBUF relus plus
the relu+add, which only ScalarE/VectorE can run, are the balanced
floor of this design (~100us end-to-end per core).
"""

import numpy as np
from contextlib import ExitStack

import concourse.bass as bass
import concourse.bacc as bacc
import concourse.tile as tile
import concourse.mybir as mybir
from concourse.bass_utils import run_bass_kernel_spmd

C = 128
N = 131072
NCORES = 8
NSH = N // NCORES          # 16384 output columns per core
T = 510                    # full-tile output width
E = T + 2                  # halo-1 intermediate width (= 512, one PSUM bank)
WH = T + 4                 # h tile width
MASK_SEED = 42
MASK_P = 0.5

F32 = mybir.dt.float32
F16 = mybir.dt.float16

LAST_RESULT = None         # BassKernelResults of the most recent run (for test.py)
TRACE = False

_mask_cache = None


def _mask_cn() -> np.ndarray:
    """The reference's fixed Bernoulli mask in [C, N] layout, float16."""
    global _mask_cache
    if _mask_cache is None:
        import jax
        cpu = jax.devices("cpu")[0]
        with jax.default_device(cpu):
            m = jax.random.bernoulli(
                jax.random.key(MASK_SEED), 1.0 - MASK_P, (1, C, N, 1)
            )
            m = np.asarray(m)[0, :, :, 0]
        _mask_cache = m.astype(np.float16)
    return _mask_cache


def _build_nc():
    nc = bacc.Bacc("TRN2", target_bir_lowering=False)

    x_t = nc.dram_tensor("x_t", [C, NSH + 4], F16, kind="ExternalInput")
    mk = nc.dram_tensor("mk", [C, NSH + 2], F16, kind="ExternalInput")
    # 11 stacked [128, 128] weight blocks, each already in lhsT ([K, M]) layout:
    # 0..2 diag(w_b1_dw taps), 3..5 diag(w_b2_dw taps),
    # 6 W1pw^T, 7 W21x1^T, 8 W2pw^T, 9 Wf[:, :C]^T, 10 Wf[:, C:]^T
    w_all = nc.dram_tensor("w_all", [11 * C, C], F16, kind="ExternalInput")
    # dw tap scalars, one per partition: row k of [6, C] = tap k (dw1 0..2, dw2 3..5)
    tp = nc.dram_tensor("tp", [6, C], F32, kind="ExternalInput")
    y_t = nc.dram_tensor("y_t", [C, NSH], F16, kind="ExternalOutput")

    D1_0, D1_1, D1_2, D2_0, D2_1, D2_2, W1PW, W21, W2PW, WFH, WFB = range(11)

    with ExitStack() as ctx:
        tc = ctx.enter_context(tile.TileContext(nc))
        wpool = ctx.enter_context(tc.tile_pool(name="weights", bufs=1))
        sb = ctx.enter_context(tc.tile_pool(name="sbuf", bufs=24))
        sbc = ctx.enter_context(tc.tile_pool(name="sbufc", bufs=10))
        ps_dw = ctx.enter_context(tc.tile_pool(name="ps_dw", bufs=2, space="PSUM"))
        ps_mm = ctx.enter_context(tc.tile_pool(name="ps_mm", bufs=1, space="PSUM"))

        w_sb = wpool.tile([C, 11 * C], F16)
        for k in range(11):
            nc.sync.dma_start(
                out=w_sb[:, k * C:(k + 1) * C], in_=w_all[k * C:(k + 1) * C, :]
            )
        tp_sb = wpool.tile([C, 6], F32)
        nc.sync.dma_start(out=tp_sb[:, :], in_=tp.rearrange("k c -> c k"))

        def w(k):
            return w_sb[:, k * C:(k + 1) * C]

        # graduated tile widths: narrow leading tiles fill the pipeline
        # sooner; steady state runs at the full 510 (PSUM-bank-limited) width
        widths = [256]
        rest = NSH - sum(widths)
        widths += [T] * (rest // T)
        if rest % T:
            widths.append(rest % T)
        assert sum(widths) == NSH
        a = 0
        for i, wT in enumerate(widths):
            wE = wT + 2
            wh = wT + 4
            la = 0

            h_c = sbc.tile([C, T + 4], F16, tag="hc")
            nc.sync.dma_start(out=h_c[:, :wh], in_=x_t[:, a:a + wh])
            mk_c = sbc.tile([C, T + 2], F16, tag="mkc")
            nc.sync.dma_start(out=mk_c[:, :wE], in_=mk[:, a:a + wE])
            o_c = sbc.tile([C, T], F16, tag="oc")
            h_t = h_c
            mk_t = mk_c

            # branch 1: depthwise taps accumulate in PSUM (PE), relu on DVE
            d1p = ps_dw.tile([C, E], F32, tag="d1", name="d1p")
            for m in range(3):
                nc.tensor.matmul(
                    d1p[:, :wE], w(D1_0 + m), h_t[:, m:m + wE],
                    start=(m == 0), stop=(m == 2),
                )
            d1s = sb.tile([C, E], F16, tag="d1s")
            nc.vector.tensor_scalar_max(d1s[:, :wE], d1p[:, :wE], 0.0)

            b1p = ps_mm.tile([C, E], F32, tag="b1", name="b1p")
            nc.tensor.matmul(b1p[:, :wE], w(W1PW), d1s[:, :wE],
                             start=True, stop=True)
            b1r = sb.tile([C, E], F16, tag="b1r")
            nc.scalar.activation(b1r[:, :wE], b1p[:, :wE],
                                 mybir.ActivationFunctionType.Relu)

            # branch 2 head: pointwise, then fused relu+add (DVE), mask (Pool)
            b2ap = ps_mm.tile([C, E], F32, tag="b2a", name="b2ap")
            nc.tensor.matmul(b2ap[:, :wE], w(W21), h_t[:, 1:1 + wE],
                             start=True, stop=True)
            # split relu+add and mask into halves so GPSIMD's left-half
            # multiply overlaps DVE's right-half relu+add
            b2b = sb.tile([C, E], F16, tag="b2b")
            b2m = sb.tile([C, E], F16, tag="b2m")
            hw_ = (wE + 7) // 8 * 2    # 4B-aligned 1/4-3/4 split: the small
                                       # first piece starts Pool sooner (DVE
                                       # piece ~530ns vs Pool's ~740ns)
            for lo, hi in ((0, hw_), (hw_, wE)):
                nc.vector.scalar_tensor_tensor(
                    b2b[:, lo:hi], b2ap[:, lo:hi], 0.0, b1r[:, lo:hi],
                    mybir.AluOpType.max, mybir.AluOpType.add,
                )
                nc.gpsimd.tensor_mul(b2m[:, lo:hi], b2b[:, lo:hi],
                                     mk_t[:, lo:hi])

            # branch 2 tail: depthwise taps (PE), relu (DVE), pointwise, relu
            d2p = ps_mm.tile([C, E], F32, tag="d2", name="d2p")
            for m in range(3):
                nc.tensor.matmul(
                    d2p[:, :wT], w(D2_0 + m), b2m[:, m:m + wT],
                    start=(m == 0), stop=(m == 2),
                )
            d2s = sb.tile([C, E], F16, tag="d2s")
            nc.vector.tensor_scalar_max(d2s[:, :wT], d2p[:, :wT], 0.0)

            b2p = ps_mm.tile([C, E], F32, tag="b2", name="b2p")
            nc.tensor.matmul(b2p[:, :wT], w(W2PW), d2s[:, :wT],
                             start=True, stop=True)
            b2r = sb.tile([C, E], F16, tag="b2r")
            nc.scalar.activation(b2r[:, :wT], b2p[:, :wT],
                                 mybir.ActivationFunctionType.Relu)

            # fusion: two accumulating matmuls, relu on DVE, store per chunk
            fp = ps_dw.tile([C, E], F32, tag="f", name="fp")
            nc.tensor.matmul(fp[:, :wT], w(WFH), h_t[:, 2:2 + wT],
                             start=True, stop=False)
            nc.tensor.matmul(fp[:, :wT], w(WFB), b2r[:, :wT],
                             start=False, stop=True)
            nc.scalar.activation(o_c[:, la:la + wT], fp[:, :wT],
                                 mybir.ActivationFunctionType.Relu)

            nc.sync.dma_start(out=y_t[:, a:a + wT], in_=o_c[:, :wT])
            a += wT

    nc.compile()
    return nc


_nc_cache = None


def kernel(x, w_b1_dw, w_b1_pw, w_b2_1x1, w_b2_dw, w_b2_pw, w_fusion):
    global LAST_RESULT, _nc_cache

    x = np.asarray(x, dtype=np.float32)
    mask = _mask_cn()

    # host-side shard prep: [C, N] layouts with zero-padded halos, fp16
    xt_pad = np.zeros((C, N + 4), dtype=np.float16)
    xt_pad[:, 2:N + 2] = x.T.astype(np.float16)
    mk_pad = np.zeros((C, N + 2), dtype=np.float16)
    mk_pad[:, 1:N + 1] = mask

    def taps(wdw):  # [C,1,3,3] -> 3 diag matrices in lhsT layout
        return [np.diag(np.asarray(wdw)[:, 0, k, 1]).astype(np.float16).T
                for k in range(3)]

    blocks = (
        taps(w_b1_dw) + taps(w_b2_dw) + [
            np.asarray(w_b1_pw)[:, :, 0, 0].T,
            np.asarray(w_b2_1x1)[:, :, 0, 0].T,
            np.asarray(w_b2_pw)[:, :, 0, 0].T,
            np.asarray(w_fusion)[:, :C, 0, 0].T,
            np.asarray(w_fusion)[:, C:, 0, 0].T,
        ]
    )
    w_all = np.ascontiguousarray(
        np.concatenate([b.astype(np.float16) for b in blocks], axis=0)
    )
    tp_arr = np.ascontiguousarray(np.concatenate([
        np.asarray(w_b1_dw)[:, 0, :, 1].T, np.asarray(w_b2_dw)[:, 0, :, 1].T,
    ], axis=0).astype(np.float32))

    in_maps = []
    for i in range(NCORES):
        s = i * NSH
        in_maps.append({
            "x_t": np.ascontiguousarray(xt_pad[:, s:s + NSH + 4]),
            "mk": np.ascontiguousarray(mk_pad[:, s:s + NSH + 2]),
            "w_all": w_all,
            "tp": tp_arr,
        })

    if _nc_cache is None:
        _nc_cache = _build_nc()

    res = run_bass_kernel_spmd(
        _nc_cache, in_maps, core_ids=list(range(NCORES)), trace=TRACE
    )
    LAST_RESULT = res

    out = np.empty((C, N), dtype=np.float32)
    for i in range(NCORES):
        out[:, i * NSH:(i + 1) * NSH] = res.results[i]["y_t"].astype(np.float32)
    return np.ascontiguousarray(out.T)

